# revision 27
# baseline (speedup 1.0000x reference)
"""Multi-head causal attention with RoPE on 8 TRN2 NeuronCores.

Sharding: batch (2) x head-groups (4 of 4 heads) -> 8 cores.
Per core, processed per 512-row s-chunk with everything interleaved to keep
the PE array dense (HAM stays at K=8/8): QKV projection for the chunk,
RoPE (stream_shuffle + sign-folded cos/sin), transposed scores
S^T = Kr @ Qr^T per head with causal block-skip, fused scale+exp from PSUM,
PV matmul with a ones-column on V accumulating the softmax denominator,
ACT-side reciprocal (exp(-ln)), ones-matmul broadcast, then the W_o partial
projection for the chunk. Host sums the 4 per-batch partials.
"""
import os
import sys

sys.path.insert(0, "/opt/trn_rl_repo")

import ml_dtypes
import numpy as np

import concourse.bass as bass
import concourse.mybir as mybir
import concourse.tile as tile
from concourse import bass_utils

F32 = mybir.dt.float32
F32R = mybir.dt.float32r
BF16 = mybir.dt.bfloat16

DT_NAME = os.environ.get("ATTN_DT", "f32r")
DT = {"f32r": F32R, "bf16": BF16}[DT_NAME]
DT_NP = {"f32r": np.float32, "bf16": ml_dtypes.bfloat16}[DT_NAME]

B, S, E, H, Dh = 2, 2048, 1024, 16, 64
HG = 4            # heads per core
HD = HG * Dh      # 256 output channels per core
SCALE = float(1.0 / np.sqrt(np.float32(1024.0)))
ROPE_BASE = 10000.0
NCHUNK = S // 512     # 4 s-chunks of 512
NTB = S // 128        # 16 t-blocks of 128
SHUF16 = list(range(16, 32)) + list(range(0, 16))

Exp = mybir.ActivationFunctionType.Exp
Ln = mybir.ActivationFunctionType.Ln
MUL = mybir.AluOpType.mult
ADD = mybir.AluOpType.add


def _build_program():
    nc = bass.Bass("TRN2", target_bir_lowering=False, debug=False)

    xT = nc.dram_tensor("xT", [E, S], DT, kind="ExternalInput")
    wq = nc.dram_tensor("wq", [E, HD], DT, kind="ExternalInput")
    wk = nc.dram_tensor("wk", [E, HD], DT, kind="ExternalInput")
    wv = nc.dram_tensor("wv", [E, HD], DT, kind="ExternalInput")
    wo = nc.dram_tensor("wo", [HD, E], DT, kind="ExternalInput")
    cosd = nc.dram_tensor("cosd", [128, S], F32, kind="ExternalInput")
    sins = nc.dram_tensor("sins", [128, S], F32, kind="ExternalInput")
    masks = nc.dram_tensor("masks", [4, 128, 512], F32, kind="ExternalInput")
    onesc = nc.dram_tensor("onesc", [128, 64], DT, kind="ExternalInput")
    sel2c = nc.dram_tensor("sel2c", [33, 128], DT, kind="ExternalInput")
    y = nc.dram_tensor("y", [S, E], F32, kind="ExternalOutput")

    with tile.TileContext(nc) as tc:
        with (
            tc.tile_pool(name="persist", bufs=1) as pp,
            tc.tile_pool(name="xchunks", bufs=(2 if DT_NAME == "bf16" else 1)) as xp,
            tc.tile_pool(name="ropetmp", bufs=(3 if DT_NAME == "bf16" else 2)) as rt,
            tc.tile_pool(name="att_es", bufs=(6 if DT_NAME == "bf16" else 3)) as ep,
            tc.tile_pool(name="att_row", bufs=2) as rp,
            tc.tile_pool(name="ystg", bufs=(2 if DT_NAME == "bf16" else 1)) as yp,
            tc.tile_pool(name="ps_proj", bufs=1, space="PSUM") as ps1,
            tc.tile_pool(name="ps_sc", bufs=2, space="PSUM") as ps_s,
            tc.tile_pool(name="ps_ot", bufs=1, space="PSUM") as ps_o,
            tc.tile_pool(name="ps_aux", bufs=1, space="PSUM") as ps_a,
        ):
            # ---- persistent tensors ----
            # Qr^T zero-padded per head half: qz[:, hi, blk, s] has rows of
            # head 2*blk+hi live and the other 64 rows zero, so scores can
            # contract over the full 128 partitions (keeps the PE array at
            # 100% activity -> HAM stays warm).
            qz = pp.tile([128, 2, 2, S], DT)
            krt = pp.tile([128, 2, S], DT)   # Kr^T
            vau = pp.tile([128, NTB, HG, 65], DT)  # V + ones col per (tb, h)
            ot = pp.tile([128, 2, S], DT)    # O^T normalized
            wo_sb = pp.tile([128, 2, E], DT)
            nc.sync.dma_start(wo_sb[:], wo.ap().rearrange("(ko p) e -> p ko e", p=128))
            ones_sb = pp.tile([128, 64], DT)
            nc.sync.dma_start(ones_sb[:], onesc.ap())
            sel2_sb = pp.tile([33, 128], DT)
            nc.sync.dma_start(sel2_sb[:], sel2c.ap())
            rows_t = pp.tile([33, 512], DT)
            mask_sb = pp.tile([128, 4, 512], F32)
            nc.sync.dma_start(mask_sb[:], masks.ap().rearrange("m p s -> p m s"))
            wq_sb = pp.tile([128, 8, HD], DT)
            nc.sync.dma_start(wq_sb[:], wq.ap().rearrange("(ko p) m -> p ko m", p=128))
            wk_sb = pp.tile([128, 8, HD], DT)
            nc.sync.dma_start(wk_sb[:], wk.ap().rearrange("(ko p) m -> p ko m", p=128))
            wv_sb = pp.tile([128, 8, HD], DT)
            nc.sync.dma_start(wv_sb[:], wv.ap().rearrange("(ko p) m -> p ko m", p=128))
            cos_sb = pp.tile([128, S], F32)
            nc.sync.dma_start(cos_sb[:], cosd.ap())
            sin_sb = pp.tile([128, S], F32)
            nc.sync.dma_start(sin_sb[:], sins.ap())

            def heat(target, n=10):
                # full-array 128x128 matmuls to trip the HAM activity window
                # back to K=8/8. Scratch lands in `target` PSUM, whose next
                # real matmul uses start=True and overwrites it.
                for _ in range(n):
                    nc.tensor.matmul(target[:, 0:128], wo_sb[:, 0, 0:128],
                                     wo_sb[:, 0, 0:128], start=True, stop=True)

            ztmp = pp.tile([128, 1], F32)
            nc.vector.memset(ztmp[:], 0.0)
            with nc.allow_low_precision(reason="rounded matmul input"):
                nc.vector.tensor_copy(rows_t[:], ztmp[0:33, :].to_broadcast((33, 512)))
            with nc.allow_low_precision(reason="rounded matmul input"):
                nc.vector.tensor_copy(qz[64:128, 0, :, :],
                                      ztmp[64:128, :].to_broadcast((64, 2, S)))
                nc.vector.tensor_copy(qz[0:64, 1, :, :],
                                      ztmp[0:64, :].to_broadcast((64, 2, S)))

            # warm the PE during the initial DMA streams
            hstart = ps_s.tile([128, 512], F32, tag="pss", name="heatstart")
            heat(hstart, n=28)

            # ones column of V_aug (free-dim broadcast from a [128,1] slice)
            with nc.allow_low_precision(reason="rounded matmul input"):
                nc.vector.tensor_copy(
                    vau[:, :, :, 64:65],
                    ones_sb[:, 0:1].to_broadcast((128, NTB, HG, 1)),
                )

            xT_r = xT.ap().rearrange("(eo p) s -> p eo s", p=128)

            for sc in range(NCHUNK):
                ss = slice(sc * 512, (sc + 1) * 512)
                ntb = 4 * sc + 4

                # ---- projection for this chunk ----
                xc = xp.tile([128, 8, 512], DT, tag="xc")
                nc.sync.dma_start(xc[:], xT_r[:, :, ss])

                # Q and K projections with RoPE
                for w_sb, dst in ((wq_sb, None), (wk_sb, krt)):
                    for mb in range(2):
                        pq = ps1.tile([128, 512], F32, tag="pq")
                        for e in range(8):
                            nc.tensor.matmul(
                                pq[:], w_sb[:, e, mb * 128:(mb + 1) * 128],
                                xc[:, e, :], start=(e == 0), stop=(e == 7),
                            )
                        a = rt.tile([128, 512], F32, tag="a")
                        nc.scalar.copy(a[:], pq[:])
                        bsh = rt.tile([128, 512], F32, tag="b")
                        nc.vector.stream_shuffle(bsh[:], a[:], SHUF16)
                        t1 = rt.tile([128, 512], F32, tag="t1")
                        nc.vector.tensor_tensor(t1[:], bsh[:], sin_sb[:, ss], MUL)
                        t2 = rt.tile([128, 512], F32, tag="t2")
                        nc.vector.tensor_tensor(t2[:], a[:], cos_sb[:, ss], MUL)
                        with nc.allow_low_precision(reason="rounded matmul input"):
                            if dst is None:  # Q: split into zero-padded halves
                                nc.vector.tensor_tensor(
                                    qz[0:64, 0, mb, ss], t2[0:64, :], t1[0:64, :], ADD)
                                nc.vector.tensor_tensor(
                                    qz[64:128, 1, mb, ss], t2[64:128, :], t1[64:128, :], ADD)
                            else:
                                nc.vector.tensor_tensor(dst[:, mb, ss], t2[:], t1[:], ADD)

                # V projection
                for tbl in range(4):
                    tb = sc * 4 + tbl
                    pv = ps1.tile([128, 256], F32, tag="pv")
                    for e in range(8):
                        nc.tensor.matmul(
                            pv[:], xc[:, e, tbl * 128:(tbl + 1) * 128],
                            wv_sb[:, e, :], start=(e == 0), stop=(e == 7),
                        )
                    with nc.allow_low_precision(reason="rounded matmul input"):
                        nc.vector.tensor_copy(
                            vau[:, tb, :, 0:64],
                            pv[:].rearrange("p (h d) -> p h d", d=64),
                        )

                # ---- attention for this chunk, one head-pair at a time ----
                for hp in range(2):
                    otp = [ps_o.tile([65, 512], F32, tag=f"ot{hi}", name=f"otp{hi}")
                           for hi in range(2)]
                    hs = (2 * hp, 2 * hp + 1)
                    for tb in range(ntb):
                        m = tb - 4 * sc
                        if sc == 3 and tb % 5 == 1:
                            htile = ps_s.tile([128, 512], F32, tag="pss", name="heatt")
                            heat(htile, n=5)
                        ess = []
                        for hi, h in enumerate(hs):
                            blk = h // 2
                            pss = ps_s.tile([128, 512], F32, tag="pss", name=f"pss{hi}")
                            nc.tensor.matmul(
                                pss[:],
                                krt[:, blk, tb * 128:(tb + 1) * 128],
                                qz[:, h % 2, blk, ss],
                                start=True, stop=True,
                            )
                            es = ep.tile([128, 512], DT, tag="es", name=f"es{hi}")
                            if m >= 0:  # diagonal block: exp then mask-multiply
                                et = ep.tile([128, 512], F32, tag="et", name=f"et{hi}")
                                nc.scalar.activation(et[:], pss[:], Exp, bias=0.0, scale=SCALE)
                                with nc.allow_low_precision(reason="rounded matmul input"):
                                    nc.vector.tensor_tensor(es[:], et[:], mask_sb[:, m, :], MUL)
                            else:
                                with nc.allow_low_precision(reason="rounded matmul input"):
                                    nc.scalar.activation(es[:], pss[:], Exp, bias=0.0, scale=SCALE)
                            ess.append(es)
                        for hi, h in enumerate(hs):
                            nc.tensor.matmul(
                                otp[hi][:], vau[:, tb, h, :], ess[hi][:],
                                start=(tb == 0), stop=(tb == ntb - 1),
                            )
                    # 1/colsum via ACT exp(-ln); both heads' rows -> [2,512],
                    # one selector matmul broadcasts to the full 128-block.
                    for hi in range(2):
                        lnr = rp.tile([1, 512], F32, tag="lnr", name=f"lnr{hi}")
                        nc.scalar.activation(lnr[:], otp[hi][64:65, :], Ln, bias=0.0, scale=1.0)
                        with nc.allow_low_precision(reason="rounded matmul input"):
                            nc.scalar.activation(rows_t[32 * hi:32 * hi + 1, :], lnr[:],
                                                 Exp, bias=0.0, scale=-1.0)
                    bc = ps_a.tile([128, 512], F32, tag="bc")
                    nc.tensor.matmul(bc[:], sel2_sb[:], rows_t[:], start=True, stop=True)
                    # normalize both heads: O rows are otp[hi][0:64]
                    for hi in range(2):
                        bcs = rp.tile([64, 512], F32, tag="bcs", name=f"bcs{hi}")
                        nc.vector.tensor_copy(bcs[:], bc[hi * 64:(hi + 1) * 64, :])
                        with nc.allow_low_precision(reason="rounded matmul input"):
                            nc.vector.tensor_tensor(ot[hi * 64:(hi + 1) * 64, hp, ss],
                                                    otp[hi][0:64, :], bcs[:], MUL)

                # ---- W_o for this chunk's 4 s-blocks ----
                for sbl in range(4):
                    sb_i = sc * 4 + sbl
                    tsl = slice(sb_i * 128, (sb_i + 1) * 128)
                    ystg = yp.tile([128, E], F32, tag="y")
                    for ec in range(2):
                        py = ps_a.tile([128, 512], F32, tag="py")
                        for blk in range(2):
                            nc.tensor.matmul(
                                py[:], ot[:, blk, tsl],
                                wo_sb[:, blk, ec * 512:(ec + 1) * 512],
                                start=(blk == 0), stop=(blk == 1),
                            )
                        nc.vector.tensor_copy(ystg[:, ec * 512:(ec + 1) * 512], py[:])
                    nc.sync.dma_start(y.ap()[tsl, :], ystg[:])

    _legalize_waits(nc)
    return nc


def _legalize_waits(nc, max_waits=1):
    """Split >max_waits sync waits onto preceding same-engine NoOps
    (several instruction encodings only have one sync-wait slot)."""
    for fn in nc.m.functions:
        for bb in fn.blocks:
            new_insts = []
            for inst in bb.instructions:
                si = inst.sync_info
                waits = list(si.on_wait) if si is not None and si.on_wait else []
                if len(waits) > max_waits:
                    carry, keep = waits[:-max_waits], waits[-max_waits:]
                    for i, w in enumerate(carry):
                        new_insts.append(mybir.InstNoOp(
                            name=f"{inst.name}_wsplit{i}",
                            engine=inst.engine,
                            bass_nofuse=True,
                            sync_info=mybir.SyncInfo(on_wait=[w], on_update=[]),
                        ))
                    si.on_wait = keep
                new_insts.append(inst)
            bb.instructions[:] = new_insts


def _host_constants():
    # RoPE channel permutation: row r (within a head, 0..63) holds source
    # channel d = 2*i + odd with i = 16*(r//32) + r%16, odd = (r%32)//16.
    r = np.arange(64)
    i_ = 16 * (r // 32) + (r % 16)
    odd = (r % 32) // 16
    dsrc = 2 * i_ + odd  # source channel per permuted row

    inv_freq = ROPE_BASE ** (-(i_.astype(np.float64)) * 2.0 / Dh)
    ang = np.arange(S, dtype=np.float64)[None, :] * inv_freq[:, None]  # [64, S]
    cos64 = np.cos(ang)
    sin64 = np.sin(ang) * np.where(odd == 0, -1.0, 1.0)[:, None]
    cosd = np.tile(cos64, (2, 1)).astype(np.float32)
    sins = np.tile(sin64, (2, 1)).astype(np.float32)

    t = np.arange(128)[None, :, None]
    s = np.arange(512)[None, None, :]
    m = np.arange(4)[:, None, None]
    masks = (m * 128 + t <= s).astype(np.float32)

    onesc = np.ones((128, 64), DT_NP)
    sel2 = np.zeros((33, 128), DT_NP)
    sel2[0, 0:64] = 1
    sel2[32, 64:128] = 1
    return dsrc, cosd, sins, masks, onesc, sel2


_CACHE = {}


def _run(inputs, trace=False):
    if "nc" not in _CACHE:
        _CACHE["nc"] = _build_program()
        _CACHE["consts"] = _host_constants()
    nc = _CACHE["nc"]
    dsrc, cosd, sins, masks, onesc, sel2 = _CACHE["consts"]

    x = np.ascontiguousarray(np.asarray(inputs["x"]), dtype=np.float32)
    W_q = np.asarray(inputs["W_q"], dtype=np.float32)
    W_k = np.asarray(inputs["W_k"], dtype=np.float32)
    W_v = np.asarray(inputs["W_v"], dtype=np.float32)
    W_o = np.asarray(inputs["W_o"], dtype=np.float32)

    xT = [np.ascontiguousarray(x[b].T).astype(DT_NP) for b in range(B)]  # [E, S]

    in_maps = []
    for c in range(8):
        b, g = divmod(c, 4)
        heads = np.arange(4 * g, 4 * g + 4)
        rows_qk = (heads[:, None] * 64 + dsrc[None, :]).reshape(-1)   # permuted
        rows_v = (heads[:, None] * 64 + np.arange(64)[None, :]).reshape(-1)
        in_maps.append({
            "xT": xT[b],
            "wq": np.ascontiguousarray(W_q[rows_qk].T).astype(DT_NP),
            "wk": np.ascontiguousarray(W_k[rows_qk].T).astype(DT_NP),
            "wv": np.ascontiguousarray(W_v[rows_v].T).astype(DT_NP),
            "wo": np.ascontiguousarray(W_o[:, rows_v].T).astype(DT_NP),
            "cosd": cosd, "sins": sins, "masks": masks, "onesc": onesc, "sel2c": sel2,
        })

    res = bass_utils.run_bass_kernel_spmd(
        nc, in_maps, core_ids=list(range(8)), trace=trace,
    )
    out = np.zeros((B, S, E), np.float32)
    for c in range(8):
        out[c // 4] += res.results[c]["y"]
    return out, res


def kernel(**inputs):
    out, _ = _run(inputs, trace=False)
    return out
